# revision 1
# baseline (speedup 1.0000x reference)
"""TRN2 Bass kernel: 16-head MHA (B=2, S=2048, H=1024) sharded over 8 NeuronCores.

Sharding: data-parallel over batch (2) x tensor-parallel over head groups
(4 groups of 4 heads). Each core computes its 4 heads' attention for its batch
and a partial output projection; the host sums the 4 partials per batch,
transposes, and adds the output bias.

Per-core kernel (all activations transposed, bf16 on-chip, fp32 accumulation):
  qhT[d,q] = wq.T @ qT ; khT likewise ; vh[k,d] = (vT.T @ wv) with a ones
  column appended per head (rowsum trick).  Scores are computed transposed
  (s^T[k,q]), exp on ScalarE (scale=1/8 folded in), multiplicative {0,1} mask
  on VectorE, and the AV matmul accumulates x^T[d+1,q] in PSUM where row 64
  is the softmax denominator.  Normalization happens after: r = 1/rowsum on
  VectorE, broadcast across partitions via a K=1 matmul, multiplied into x.
"""

import sys

sys.path.insert(0, "/opt/trn_rl_repo")

from contextlib import ExitStack

import numpy as np
import ml_dtypes

import concourse.tile as tile
from concourse import bacc, mybir

BF16 = mybir.dt.bfloat16
F32 = mybir.dt.float32
F32R = mybir.dt.float32r
P = 128

_PROGRAM_CACHE = {}


def build_mha_program(S=2048, HID=1024, NH=4, DK=64, QB=1024, aug=False):
    """Build + compile the per-core SPMD Bass program."""
    D = NH * DK
    assert NH % 2 == 0 and DK == 64
    SH = S // P
    HT = HID // P
    HTa = HT + (1 if aug else 0)
    QBn = S // QB
    NS = min(512, QB)
    QH = QB // NS
    NQ = S // NS
    DC = D // P
    NPAIR = NH // 2
    GW = DK + 2                  # 64 data cols + rowsum-ones col + pad (4B-aligned groups)

    nc = bacc.Bacc("TRN2", target_bir_lowering=False, debug=False)

    qT_d = nc.dram_tensor("qT", [HTa * P, S], BF16, kind="ExternalInput").ap()
    kT_d = nc.dram_tensor("kT", [HTa * P, S], BF16, kind="ExternalInput").ap()
    vT_d = nc.dram_tensor("vT", [HTa * P, S], BF16, kind="ExternalInput").ap()
    maskT_d = nc.dram_tensor("maskT", [S, S], BF16, kind="ExternalInput").ap()
    wq_d = nc.dram_tensor("wq", [HTa * P, D], BF16, kind="ExternalInput").ap()
    wk_d = nc.dram_tensor("wk", [HTa * P, D], BF16, kind="ExternalInput").ap()
    wv_d = nc.dram_tensor("wv", [HTa * P, D], BF16, kind="ExternalInput").ap()
    wo_d = nc.dram_tensor("wo", [D, HID], BF16, kind="ExternalInput").ap()
    y_d = nc.dram_tensor("y", [HID, S], F32, kind="ExternalOutput").ap()
    # DRAM bounce buffer for partition-broadcasting the softmax reciprocals
    rb_d = nc.dram_tensor("r_bounce", [32 * QBn, QB], F32).ap()

    Exp = mybir.ActivationFunctionType.Exp

    with tile.TileContext(nc) as tc:
        with ExitStack() as ctx:
            persist = ctx.enter_context(tc.tile_pool(name="persist", bufs=1))
            qh_t = [persist.tile([P, S], BF16, tag=f"qh{d}", name=f"qh{d}")
                    for d in range(DC)]
            kh_t = [persist.tile([P, S], BF16, tag=f"kh{d}", name=f"kh{d}")
                    for d in range(DC)]
            vh_t = [persist.tile([P, NH * GW], BF16, tag=f"vh{s}", name=f"vh{s}")
                    for s in range(SH)]
            xu_t = [persist.tile([P, S], BF16, tag=f"xu{p}", name=f"xu{p}")
                    for p in range(NPAIR)]
            xn_t = [persist.tile([P, S], BF16, tag=f"xn{p}", name=f"xn{p}")
                    for p in range(NPAIR)]
            wo_t = [persist.tile([P, HID], BF16, tag=f"wo{p}", name=f"wo{p}")
                    for p in range(NPAIR)]
            rs_t = persist.tile([32 * QBn, QB], F32, tag="rs", name="rs")
            r_t = persist.tile([32 * QBn, QB], F32, tag="r", name="r")

            for pr in range(NPAIR):
                nc.sync.dma_start(wo_t[pr][:], wo_d[pr * P:(pr + 1) * P, :])

            # phase 1: projections
            with ExitStack() as ph1:
                inp = ph1.enter_context(tc.tile_pool(name="inp", bufs=1))
                wp = ph1.enter_context(tc.tile_pool(name="wp", bufs=1))
                ps1 = ph1.enter_context(
                    tc.tile_pool(name="ps1", bufs=1, space="PSUM"))

                qT_t = [inp.tile([P, S], BF16, tag=f"qT{i}", name=f"qT{i}")
                        for i in range(HTa)]
                kT_t = [inp.tile([P, S], BF16, tag=f"kT{i}", name=f"kT{i}")
                        for i in range(HTa)]
                vT_t = [inp.tile([P, S], BF16, tag=f"vT{i}", name=f"vT{i}")
                        for i in range(HTa)]
                wq_t = [wp.tile([P, D], BF16, tag=f"wq{i}", name=f"wq{i}")
                        for i in range(HTa)]
                wk_t = [wp.tile([P, D], BF16, tag=f"wk{i}", name=f"wk{i}")
                        for i in range(HTa)]
                wv_t = [wp.tile([P, D], BF16, tag=f"wv{i}", name=f"wv{i}")
                        for i in range(HTa)]
                for i in range(HTa):
                    sl = slice(i * P, (i + 1) * P)
                    nc.sync.dma_start(qT_t[i][:], qT_d[sl, :])
                    nc.sync.dma_start(kT_t[i][:], kT_d[sl, :])
                    nc.sync.dma_start(vT_t[i][:], vT_d[sl, :])
                    nc.sync.dma_start(wq_t[i][:], wq_d[sl, :])
                    nc.sync.dma_start(wk_t[i][:], wk_d[sl, :])
                    nc.sync.dma_start(wv_t[i][:], wv_d[sl, :])

                for (src_t, w_t, dst) in ((qT_t, wq_t, qh_t),
                                          (kT_t, wk_t, kh_t)):
                    for dc in range(DC):
                        psl = [ps1.tile([P, NS], F32, tag=f"p1_{qc}",
                                        name=f"p1_{qc}") for qc in range(NQ)]
                        for i in range(HTa):
                            for qc in range(NQ):
                                nc.tensor.matmul(
                                    psl[qc][:],
                                    w_t[i][:, dc * P:(dc + 1) * P],
                                    src_t[i][:, qc * NS:(qc + 1) * NS],
                                    start=(i == 0), stop=(i == HTa - 1))
                        for qc in range(NQ):
                            nc.vector.tensor_copy(
                                dst[dc][:, qc * NS:(qc + 1) * NS], psl[qc][:])

                # vh[k, d]: direct projection, ones cols (rowsum trick) from
                # the memset survive the grouped copy
                for sc in range(SH):
                    ps_v = ps1.tile([P, D], F32, tag="p1v", name="p1v", bufs=2)
                    for i in range(HTa):
                        nc.tensor.matmul(
                            ps_v[:],
                            vT_t[i][:, sc * P:(sc + 1) * P],
                            wv_t[i][:],
                            start=(i == 0), stop=(i == HTa - 1))
                    nc.vector.memset(vh_t[sc][:], 1.0)
                    dst_v = vh_t[sc][:].rearrange(
                        "p (h c) -> p h c", c=GW)[:, :, 0:DK]
                    src_v = ps_v[:].rearrange("p (h c) -> p h c", c=DK)
                    nc.vector.tensor_copy(dst_v, src_v)

            # phase 2+3+4 fused: attention per (q-block, head); after each
            # q-block completes, its normalize + output projection are
            # emitted interleaved into the NEXT q-block's attention as PE
            # filler work (the attention pipeline is ScalarE-bound).
            with ExitStack() as ph2:
                mp = ph2.enter_context(tc.tile_pool(name="mask", bufs=1))
                pp = ph2.enter_context(tc.tile_pool(name="pexp", bufs=3))
                pmp = ph2.enter_context(tc.tile_pool(name="pmask", bufs=8))
                stg = ph2.enter_context(tc.tile_pool(name="stg", bufs=2))
                rbp = ph2.enter_context(tc.tile_pool(name="rbp", bufs=3))
                ysb = ph2.enter_context(tc.tile_pool(name="ysb", bufs=4))
                sps = ph2.enter_context(
                    tc.tile_pool(name="sps", bufs=3, space="PSUM"))
                xps = ph2.enter_context(
                    tc.tile_pool(name="xps", bufs=1, space="PSUM"))

                mask_t = [mp.tile([P, S], BF16, tag=f"m{i}", name=f"m{i}")
                          for i in range(SH)]
                for i in range(SH):
                    nc.scalar.dma_start(mask_t[i][:],
                                        maskT_d[i * P:(i + 1) * P, :])

                def attention_head(qb, h):
                    """QK -> exp -> mask -> AV (deep-lagged) for one head.

                    AV matmuls are emitted LAG chunks late so their semaphore
                    waits never block the PE (a blocking PE wait starts a
                    vicious cycle: PE idles, HAM drops it to 1.2 GHz, the
                    slowed pipeline then blocks on every wait)."""
                    LAG = 6
                    qsl = slice(qb * QB, (qb + 1) * QB)
                    ht, hb = divmod(h, 2)
                    hsl = slice(64 * hb, 64 * hb + 64)
                    x_ps = xps.tile([P, QB], F32, tag="x", name="x")
                    pending = []

                    def emit_av(kc, pm_t):
                        for qh_ in range(QH):
                            nsl = slice(qh_ * NS, (qh_ + 1) * NS)
                            nc.tensor.matmul(
                                x_ps[:DK + 1, nsl],
                                vh_t[kc][:, h * GW:h * GW + DK + 1],
                                pm_t[:, nsl],
                                start=(kc == 0), stop=(kc == SH - 1),
                                skip_group_check=True)

                    for kc in range(SH):
                        s_ps = sps.tile([P, QB], F32, tag="s", name="s")
                        for qh_ in range(QH):
                            nsl = slice(qh_ * NS, (qh_ + 1) * NS)
                            nc.tensor.matmul(
                                s_ps[:, nsl],
                                kh_t[ht][hsl, kc * P:(kc + 1) * P],
                                qh_t[ht][hsl, qb * QB + qh_ * NS:
                                         qb * QB + (qh_ + 1) * NS],
                                start=True, stop=True)
                        p_t = pp.tile([P, QB], BF16, tag="p", name="p")
                        nc.scalar.activation(p_t[:], s_ps[:], Exp, scale=0.125)
                        pm_t = pmp.tile([P, QB], BF16, tag="pm", name="pm")
                        nc.vector.tensor_mul(
                            pm_t[:], p_t[:], mask_t[kc][:, qsl])
                        pending.append((kc, pm_t))
                        if len(pending) > LAG:
                            emit_av(*pending.pop(0))
                    for item in pending:
                        emit_av(*item)

                    row = qb * 32 + h
                    stage = stg.tile([GW, QB], F32, tag="stg", name="stg")
                    nc.vector.tensor_copy(stage[DK:DK + 1, :],
                                          x_ps[DK:DK + 1, :])
                    nc.sync.dma_start(rs_t[row:row + 1, :],
                                      stage[DK:DK + 1, :])
                    nc.vector.tensor_copy(xu_t[ht][hsl, qsl], x_ps[:DK, :])

                def normalize_qblock(qb):
                    """reciprocal of this q-block's rowsums, partition
                    broadcast via DRAM round-trip, then xn = xu * r."""
                    qsl = slice(qb * QB, (qb + 1) * QB)
                    rows = slice(qb * 32, qb * 32 + NH)
                    nc.vector.tensor_scalar_max(rs_t[rows, :], rs_t[rows, :],
                                                1e-30)
                    nc.vector.reciprocal(r_t[rows, :], rs_t[rows, :])
                    nc.sync.dma_start(rb_d[rows, :], r_t[rows, :])
                    for pr in range(NPAIR):
                        rb = rbp.tile([P, QB], F32, tag="rb", name="rb")
                        for hb in range(2):
                            row = qb * 32 + 2 * pr + hb
                            eng = (nc.sync, nc.scalar)[(pr + hb) % 2]
                            eng.dma_start(
                                rb[64 * hb:64 * hb + 64, :],
                                rb_d[row:row + 1, :].broadcast_to([64, QB]))
                        nc.vector.tensor_mul(
                            xn_t[pr][:, qsl], xu_t[pr][:, qsl], rb[:])

                def oproj_chunk(qb, hc):
                    """y[hc, qb] = sum over pairs wo^T @ xn, via an s-pool
                    PSUM slot; runs as PE filler inside later q-blocks."""
                    for qh_ in range(QH):
                        y_ps = sps.tile([P, NS], F32, tag="s", name="yps")
                        qc0 = qb * QH + qh_
                        for pr in range(NPAIR):
                            nc.tensor.matmul(
                                y_ps[:],
                                wo_t[pr][:, hc * P:(hc + 1) * P],
                                xn_t[pr][:, qc0 * NS:(qc0 + 1) * NS],
                                start=(pr == 0), stop=(pr == NPAIR - 1))
                        y_sb = ysb.tile([P, NS], F32, tag="ysb", name="ysb")
                        nc.vector.tensor_copy(y_sb[:], y_ps[:])
                        (nc.sync, nc.scalar, nc.gpsimd)[qc0 % 3].dma_start(
                            y_d[hc * P:(hc + 1) * P,
                                qc0 * NS:(qc0 + 1) * NS],
                            y_sb[:])

                hc_groups = [[hc for hc in range(HT) if hc % NH == h]
                             for h in range(NH)]
                for qb in range(QBn):
                    for h in range(NH):
                        attention_head(qb, h)
                        if qb >= 1:
                            for hc in hc_groups[h]:
                                oproj_chunk(qb - 1, hc)
                    normalize_qblock(qb)
                for hc in range(HT):
                    oproj_chunk(QBn - 1, hc)

    nc.compile()
    return nc


def make_in_maps(q, k, v, mask, Wq, bq, Wk, bk, Wv, bv, Wo,
                 n_cores=8, NH=4, DK=64, aug=False):
    bf = ml_dtypes.bfloat16
    B, S, HID = q.shape
    D = NH * DK
    n_hg = n_cores // B

    def with_aug(xT, bias_row):
        pad = np.zeros((P, xT.shape[1]), xT.dtype)
        pad[0, :] = bias_row
        return np.concatenate([xT, pad], axis=0)

    per_batch = {}
    for b in range(B):
        qT = np.ascontiguousarray(q[b].T).astype(bf)
        kT = np.ascontiguousarray(k[b].T).astype(bf)
        vT = np.ascontiguousarray(v[b].T).astype(bf)
        if aug:
            one = np.ones((S,), np.float32).astype(bf)
            qT, kT, vT = with_aug(qT, one), with_aug(kT, one), with_aug(vT, one)
        per_batch[b] = (qT, kT, vT,
                        np.ascontiguousarray(mask[b, 0].T != 0).astype(bf))

    in_maps = []
    for core in range(n_cores):
        b, hg = divmod(core, n_hg)
        hsl = slice(hg * D, (hg + 1) * D)
        wq = Wq[:, hsl].astype(bf)
        wk = Wk[:, hsl].astype(bf)
        wv = Wv[:, hsl].astype(bf)
        if aug:
            wq = with_aug(wq, bq[hsl].astype(bf))
            wk = with_aug(wk, bk[hsl].astype(bf))
            wv = with_aug(wv, bv[hsl].astype(bf))
        qT, kT, vT, mT = per_batch[b]
        in_maps.append(dict(
            qT=qT, kT=kT, vT=vT, maskT=mT,
            wq=np.ascontiguousarray(wq), wk=np.ascontiguousarray(wk),
            wv=np.ascontiguousarray(wv),
            wo=np.ascontiguousarray(Wo[hsl, :]).astype(bf),
        ))
    return in_maps


def combine_outputs(results, B, S, HID, bo, n_cores=8):
    n_hg = n_cores // B
    out = np.zeros((B, S, HID), np.float32)
    for core in range(n_cores):
        b = core // n_hg
        out[b] += results[core]["y"].T
    return out + bo.astype(np.float32)


def run_mha(q, k, v, mask, Wq, bq, Wk, bk, Wv, bv, Wo, bo, trace=False):
    from concourse.bass_utils import run_bass_kernel_spmd

    B, S, HID = q.shape
    n_cores = 8
    aug = bool(np.any(bq) or np.any(bk) or np.any(bv))
    key = (S, HID, aug)
    if key not in _PROGRAM_CACHE:
        _PROGRAM_CACHE[key] = build_mha_program(S=S, HID=HID, aug=aug)
    nc = _PROGRAM_CACHE[key]
    in_maps = make_in_maps(q, k, v, mask, Wq, bq, Wk, bk, Wv, bv, Wo,
                           n_cores=n_cores, aug=aug)
    res = run_bass_kernel_spmd(nc, in_maps, list(range(n_cores)), trace=trace)
    out = combine_outputs(res.results, B, S, HID, bo, n_cores=n_cores)
    return out, res


def kernel(q, k, v, mask, Wq, bq, Wk, bk, Wv, bv, Wo, bo):
    q = np.asarray(q, np.float32)
    k = np.asarray(k, np.float32)
    v = np.asarray(v, np.float32)
    mask = np.asarray(mask)
    out, _ = run_mha(q, k, v, mask,
                     np.asarray(Wq, np.float32), np.asarray(bq, np.float32),
                     np.asarray(Wk, np.float32), np.asarray(bk, np.float32),
                     np.asarray(Wv, np.float32), np.asarray(bv, np.float32),
                     np.asarray(Wo, np.float32), np.asarray(bo, np.float32))
    return out



# revision 5
# speedup vs baseline: 1.2553x; 1.2553x over previous
"""TRN2 Bass kernel: 16-head MHA (B=2, S=2048, H=1024) sharded over 8 NeuronCores.

Sharding: data-parallel over batch (2) x tensor-parallel over head groups
(4 groups of 4 heads). Each core computes its 4 heads' attention for its batch
and a partial output projection; the host sums the 4 partials per batch,
transposes, and adds the output bias.

v2 redesign (pair-packed PE tiles):
  - QK^T per head uses only K=64 of the 128 contraction rows; heads of a pair
    are issued back-to-back as row-tiles (tile_position (0,0)/(64,0)) so both
    run concurrently in the systolic array -> ~2x QK throughput.
  - AV per head uses only M=64 output columns; the pair is issued as col-tiles
    ((0,0)/(0,64)) writing partitions 0-63 / 64-127 of one PSUM bank.
  - Softmax denominators come from a 4-way col-tiled pass of M=1 ones-matmuls
    (rows 0/32/64/96 of one PSUM bank), replacing the ones-column vh hack.
  - exp() is one [128, 1024] ACTIVATE per (qb, kc, pair) spanning 2 PSUM banks.
  - 1/rowsum via reciprocal_approx_fast (~5x faster than DVE reciprocal);
    partition-broadcast via a bf16 DRAM round trip.
  - Phase-1 PSUM evacuations run on ScalarE (idle during projections).
  - Inputs load as few big multi-engine DMAs; y is written bf16 (host sums
    partials in fp32).
"""

import sys

sys.path.insert(0, "/opt/trn_rl_repo")

from collections import deque
from contextlib import ExitStack

import numpy as np
import ml_dtypes

import concourse.tile as tile
from concourse import bacc, mybir

BF16 = mybir.dt.bfloat16
F32 = mybir.dt.float32
P = 128

LAG = 5            # kc-instances by which AV/rowsum matmuls trail QK/exp/mask
USE_GPSIMD_MASK = True   # offload 1/4 of mask multiplies to GPSIMD
OPROJ_EVERY = 2    # pop one oproj chunk every N kc-instances

_PROGRAM_CACHE = {}


def build_mha_program(S=2048, HID=1024, NH=4, DK=64, QB=512, aug=False):
    """Build + compile the per-core SPMD Bass program."""
    D = NH * DK
    assert NH == 4 and DK == 64
    SH = S // P                 # 16 key blocks
    HT = HID // P               # 8 hidden blocks
    HTa = HT + (1 if aug else 0)
    QBn = S // QB               # 4 q-blocks
    NPAIR = NH // 2             # 2 head pairs
    NS1 = 512                   # phase-1 psum chunk
    NQ1 = S // NS1

    nc = bacc.Bacc("TRN2", target_bir_lowering=False, debug=False)

    qT_d = nc.dram_tensor("qT", [HTa * P, S], BF16, kind="ExternalInput").ap()
    kT_d = nc.dram_tensor("kT", [HTa * P, S], BF16, kind="ExternalInput").ap()
    vT_d = nc.dram_tensor("vT", [HTa * P, S], BF16, kind="ExternalInput").ap()
    maskT_d = nc.dram_tensor("maskT", [S, S], BF16, kind="ExternalInput").ap()
    wq_d = nc.dram_tensor("wq", [HTa * P, D], BF16, kind="ExternalInput").ap()
    wk_d = nc.dram_tensor("wk", [HTa * P, D], BF16, kind="ExternalInput").ap()
    wv_d = nc.dram_tensor("wv", [HTa * P, D], BF16, kind="ExternalInput").ap()
    wo_d = nc.dram_tensor("wo", [D, HID], BF16, kind="ExternalInput").ap()
    y_d = nc.dram_tensor("y", [HID, S], BF16, kind="ExternalOutput").ap()
    # DRAM bounce buffer for partition-broadcasting the softmax reciprocals
    rb_d = nc.dram_tensor("r_bounce", [NH * QBn, QB], BF16).ap()

    Exp = mybir.ActivationFunctionType.Exp

    with tile.TileContext(nc) as tc:
        with ExitStack() as ctx:
            persist = ctx.enter_context(tc.tile_pool(name="persist", bufs=1))
            qh_t = [persist.tile([P, S], BF16, tag=f"qh{d}", name=f"qh{d}")
                    for d in range(NPAIR)]
            kh_t = [persist.tile([P, S], BF16, tag=f"kh{d}", name=f"kh{d}")
                    for d in range(NPAIR)]
            vh_t = [persist.tile([P, D], BF16, tag=f"vh{s}", name=f"vh{s}")
                    for s in range(SH)]
            xu_t = [persist.tile([P, S], BF16, tag=f"xu{p}", name=f"xu{p}")
                    for p in range(NPAIR)]
            xn_t = [persist.tile([P, S], BF16, tag=f"xn{p}", name=f"xn{p}")
                    for p in range(NPAIR)]
            wo_t = [persist.tile([P, HID], BF16, tag=f"wo{p}", name=f"wo{p}")
                    for p in range(NPAIR)]
            ones_t = persist.tile([P, 4], BF16, tag="ones", name="ones")
            nc.vector.memset(ones_t[:], 1.0)

            nc.gpsimd.dma_start(wo_t[0][:], wo_d[0:P, :])
            nc.gpsimd.dma_start(wo_t[1][:], wo_d[P:2 * P, :])

            # ---------------- phase 1: projections ----------------
            with ExitStack() as ph1:
                inp = ph1.enter_context(tc.tile_pool(name="inp", bufs=1))
                ps1 = ph1.enter_context(
                    tc.tile_pool(name="ps1", bufs=1, space="PSUM"))

                qT_t = inp.tile([P, HTa * S], BF16, tag="qT", name="qT")
                kT_t = inp.tile([P, HTa * S], BF16, tag="kT", name="kT")
                vT_t = inp.tile([P, HTa * S], BF16, tag="vT", name="vT")
                wq_t = inp.tile([P, HTa * D], BF16, tag="wq", name="wq")
                wk_t = inp.tile([P, HTa * D], BF16, tag="wk", name="wk")
                wv_t = inp.tile([P, HTa * D], BF16, tag="wv", name="wv")

                def big_load(eng, dst_t, src_d, i0, i1, w):
                    """Load row-blocks [i0, i1) of src [HTa*P, w] into dst."""
                    dst = dst_t[:, i0 * w:i1 * w].rearrange(
                        "p (i s) -> p i s", s=w)
                    src = src_d[i0 * P:i1 * P, :].rearrange(
                        "(i p) s -> p i s", p=P)
                    eng.dma_start(dst, src)

                hh = HTa // 2
                big_load(nc.gpsimd, wq_t, wq_d, 0, HTa, D)
                big_load(nc.gpsimd, wk_t, wk_d, 0, HTa, D)
                big_load(nc.gpsimd, wv_t, wv_d, 0, HTa, D)
                big_load(nc.sync, qT_t, qT_d, 0, hh, S)
                big_load(nc.scalar, qT_t, qT_d, hh, HTa, S)
                big_load(nc.gpsimd, kT_t, kT_d, 0, hh, S)
                big_load(nc.sync, kT_t, kT_d, hh, HTa, S)
                big_load(nc.scalar, vT_t, vT_d, 0, hh, S)
                big_load(nc.gpsimd, vT_t, vT_d, hh, HTa, S)

                for (src_t, w_t, dst) in ((qT_t, wq_t, qh_t),
                                          (kT_t, wk_t, kh_t)):
                    for dc in range(NPAIR):
                        psl = [ps1.tile([P, NS1], F32, tag=f"p1_{qc}",
                                        name=f"p1_{qc}") for qc in range(NQ1)]
                        for i in range(HTa):
                            for qc in range(NQ1):
                                nc.tensor.matmul(
                                    psl[qc][:],
                                    w_t[:, i * D + dc * P:i * D + (dc + 1) * P],
                                    src_t[:, i * S + qc * NS1:
                                          i * S + (qc + 1) * NS1],
                                    start=(i == 0), stop=(i == HTa - 1))
                        for qc in range(NQ1):
                            nc.scalar.copy(
                                dst[dc][:, qc * NS1:(qc + 1) * NS1], psl[qc][:])

                for sc in range(SH):
                    ps_v = ps1.tile([P, NS1], F32, tag="p1v", name="p1v",
                                    bufs=2)
                    for i in range(HTa):
                        nc.tensor.matmul(
                            ps_v[:, 0:D],
                            vT_t[:, i * S + sc * P:i * S + (sc + 1) * P],
                            wv_t[:, i * D:(i + 1) * D],
                            start=(i == 0), stop=(i == HTa - 1))
                    nc.scalar.copy(vh_t[sc][:], ps_v[:, 0:D])

            # ---------------- phase 2: attention ----------------
            with ExitStack() as ph2:
                mp = ph2.enter_context(tc.tile_pool(name="mask", bufs=1))
                pp = ph2.enter_context(tc.tile_pool(name="pexp", bufs=3))
                pmp = ph2.enter_context(
                    tc.tile_pool(name="pmask", bufs=2 * (LAG + 2)))
                rfp = ph2.enter_context(tc.tile_pool(name="rfp", bufs=2))
                rbp = ph2.enter_context(tc.tile_pool(name="rbp", bufs=4))
                ysb = ph2.enter_context(tc.tile_pool(name="ysb", bufs=3))
                sps = ph2.enter_context(
                    tc.tile_pool(name="sps", bufs=2, space="PSUM"))
                xps = ph2.enter_context(
                    tc.tile_pool(name="xps", bufs=1, space="PSUM"))
                rsps = ph2.enter_context(
                    tc.tile_pool(name="rsps", bufs=1, space="PSUM"))
                yps = ph2.enter_context(
                    tc.tile_pool(name="yps", bufs=1, space="PSUM"))

                mask_t = [mp.tile([P, S], BF16, tag=f"m{i}", name=f"m{i}")
                          for i in range(SH)]
                mask_engs = (nc.sync, nc.gpsimd)
                for i in range(SH):
                    mask_engs[i % 2].dma_start(mask_t[i][:],
                                               maskT_d[i * P:(i + 1) * P, :])

                x_ps = [xps.tile([P, QB], F32, tag=f"x{p}", name=f"x{p}")
                        for p in range(NPAIR)]
                rs_ps = rsps.tile([P, QB], F32, tag="rs", name="rs")

                def emit_qk_exp_mask(qb, kc, t):
                    """Pair-packed QK (row tiles), wide exp, mask multiply."""
                    qsl = slice(qb * QB, (qb + 1) * QB)
                    pms = []
                    for pr in range(NPAIR):
                        s_ps = sps.tile([P, 2 * QB], F32, tag="s", name="s")
                        for hb in range(2):
                            rsl = slice(64 * hb, 64 * hb + 64)
                            nc.tensor.matmul(
                                s_ps[:, hb * QB:(hb + 1) * QB],
                                kh_t[pr][rsl, kc * P:(kc + 1) * P],
                                qh_t[pr][rsl, qsl],
                                start=True, stop=True)
                        p_t = pp.tile([P, 2 * QB], BF16, tag="p", name="p")
                        nc.scalar.activation(p_t[:], s_ps[:], Exp, scale=0.125)
                        pm = pmp.tile([P, 2 * QB], BF16, tag="pm", name="pm")
                        eng = nc.vector
                        if USE_GPSIMD_MASK and (pr == 1) and (kc % 2 == 1):
                            eng = nc.gpsimd
                        for hb in range(2):
                            eng.tensor_mul(
                                pm[:, hb * QB:(hb + 1) * QB],
                                p_t[:, hb * QB:(hb + 1) * QB],
                                mask_t[kc][:, qsl])
                        pms.append(pm)
                    return pms

                def emit_av_rs(qb, kc, pms):
                    """Col-tiled AV pair + 4-way col-tiled rowsum quad."""
                    for pr in range(NPAIR):
                        for hb in range(2):
                            h = 2 * pr + hb
                            nc.tensor.matmul(
                                x_ps[pr][64 * hb:64 * hb + 64, :],
                                vh_t[kc][:, h * DK:(h + 1) * DK],
                                pms[pr][:, hb * QB:(hb + 1) * QB],
                                start=(kc == 0), stop=(kc == SH - 1),
                                skip_group_check=True)
                    for pr in range(NPAIR):
                        for hb in range(2):
                            h = 2 * pr + hb
                            nc.tensor.matmul(
                                rs_ps[32 * h:32 * h + 1, :],
                                ones_t[:, 0:1],
                                pms[pr][:, hb * QB:(hb + 1) * QB],
                                start=(kc == 0), stop=(kc == SH - 1),
                                skip_group_check=True,
                                tile_position=(0, 32 * h))

                def normalize(qb):
                    """xu <- x_ps; r = 1/rowsum; broadcast via DRAM; xn."""
                    qsl = slice(qb * QB, (qb + 1) * QB)
                    for pr in range(NPAIR):
                        nc.vector.tensor_copy(xu_t[pr][:, qsl], x_ps[pr][:])
                    r32 = rfp.tile([P, QB], F32, tag="r32", name="r32")
                    r16 = rfp.tile([P, QB], BF16, tag="r16", name="r16")
                    nc.vector.reciprocal_approx_fast(out=r32[:], in_=rs_ps[:])
                    nc.vector.tensor_copy(r16[:], r32[:])
                    rows = r16[:].rearrange("(g p) q -> g p q", p=32)[:, 0:1, :]
                    nc.sync.dma_start(
                        rb_d[qb * NH:(qb + 1) * NH, :].rearrange(
                            "(g o) q -> g o q", o=1),
                        rows)
                    for pr in range(NPAIR):
                        rb = rbp.tile([P, QB], BF16, tag="rb", name="rb")
                        for hb in range(2):
                            row = qb * NH + 2 * pr + hb
                            nc.sync.dma_start(
                                rb[64 * hb:64 * hb + 64, :],
                                rb_d[row:row + 1, :].broadcast_to([64, QB]))
                        nc.vector.tensor_mul(
                            xn_t[pr][:, qsl], xu_t[pr][:, qsl], rb[:])

                def oproj_chunk(qb, hc):
                    """y[hc, qb] = sum over pairs wo^T @ xn."""
                    qsl = slice(qb * QB, (qb + 1) * QB)
                    y_ps = yps.tile([P, QB], F32, tag="y", name="y")
                    for pr in range(NPAIR):
                        nc.tensor.matmul(
                            y_ps[:],
                            wo_t[pr][:, hc * P:(hc + 1) * P],
                            xn_t[pr][:, qsl],
                            start=(pr == 0), stop=(pr == NPAIR - 1))
                    y_sb = ysb.tile([P, QB], BF16, tag="ysb", name="ysb")
                    nc.vector.tensor_copy(y_sb[:], y_ps[:])
                    nc.sync.dma_start(y_d[hc * P:(hc + 1) * P, qsl], y_sb[:])

                pending = deque()
                oproj_q = deque()

                def pop_pending():
                    qb0, kc0, pms0 = pending.popleft()
                    emit_av_rs(qb0, kc0, pms0)
                    if kc0 == SH - 1:
                        normalize(qb0)
                        for hc in range(HT):
                            oproj_q.append((qb0, hc))

                t = 0
                for qb in range(QBn):
                    for kc in range(SH):
                        pms = emit_qk_exp_mask(qb, kc, t)
                        pending.append((qb, kc, pms))
                        if len(pending) > LAG:
                            pop_pending()
                        if t % OPROJ_EVERY == 1 and oproj_q:
                            oproj_chunk(*oproj_q.popleft())
                        t += 1
                while pending:
                    pop_pending()
                while oproj_q:
                    oproj_chunk(*oproj_q.popleft())

    nc.compile()
    return nc


def make_in_maps(q, k, v, mask, Wq, bq, Wk, bk, Wv, bv, Wo,
                 n_cores=8, NH=4, DK=64, aug=False):
    bf = ml_dtypes.bfloat16
    B, S, HID = q.shape
    D = NH * DK
    n_hg = n_cores // B

    def with_aug(xT, bias_row):
        pad = np.zeros((P, xT.shape[1]), xT.dtype)
        pad[0, :] = bias_row
        return np.concatenate([xT, pad], axis=0)

    per_batch = {}
    for b in range(B):
        qT = np.ascontiguousarray(q[b].T).astype(bf)
        kT = np.ascontiguousarray(k[b].T).astype(bf)
        vT = np.ascontiguousarray(v[b].T).astype(bf)
        if aug:
            one = np.ones((S,), np.float32).astype(bf)
            qT, kT, vT = with_aug(qT, one), with_aug(kT, one), with_aug(vT, one)
        per_batch[b] = (qT, kT, vT,
                        np.ascontiguousarray(mask[b, 0].T != 0).astype(bf))

    in_maps = []
    for core in range(n_cores):
        b, hg = divmod(core, n_hg)
        hsl = slice(hg * D, (hg + 1) * D)
        wq = Wq[:, hsl].astype(bf)
        wk = Wk[:, hsl].astype(bf)
        wv = Wv[:, hsl].astype(bf)
        if aug:
            wq = with_aug(wq, bq[hsl].astype(bf))
            wk = with_aug(wk, bk[hsl].astype(bf))
            wv = with_aug(wv, bv[hsl].astype(bf))
        qT, kT, vT, mT = per_batch[b]
        in_maps.append(dict(
            qT=qT, kT=kT, vT=vT, maskT=mT,
            wq=np.ascontiguousarray(wq), wk=np.ascontiguousarray(wk),
            wv=np.ascontiguousarray(wv),
            wo=np.ascontiguousarray(Wo[hsl, :]).astype(bf),
        ))
    return in_maps


def combine_outputs(results, B, S, HID, bo, n_cores=8):
    n_hg = n_cores // B
    out = np.zeros((B, S, HID), np.float32)
    for core in range(n_cores):
        b = core // n_hg
        out[b] += results[core]["y"].astype(np.float32).T
    return out + bo.astype(np.float32)


def run_mha(q, k, v, mask, Wq, bq, Wk, bk, Wv, bv, Wo, bo, trace=False):
    from concourse.bass_utils import run_bass_kernel_spmd

    B, S, HID = q.shape
    n_cores = 8
    aug = bool(np.any(bq) or np.any(bk) or np.any(bv))
    key = (S, HID, aug)
    if key not in _PROGRAM_CACHE:
        _PROGRAM_CACHE[key] = build_mha_program(S=S, HID=HID, aug=aug)
    nc = _PROGRAM_CACHE[key]
    in_maps = make_in_maps(q, k, v, mask, Wq, bq, Wk, bk, Wv, bv, Wo,
                           n_cores=n_cores, aug=aug)
    res = run_bass_kernel_spmd(nc, in_maps, list(range(n_cores)), trace=trace)
    out = combine_outputs(res.results, B, S, HID, bo, n_cores=n_cores)
    return out, res


def kernel(q, k, v, mask, Wq, bq, Wk, bk, Wv, bv, Wo, bo):
    q = np.asarray(q, np.float32)
    k = np.asarray(k, np.float32)
    v = np.asarray(v, np.float32)
    mask = np.asarray(mask)
    out, _ = run_mha(q, k, v, mask,
                     np.asarray(Wq, np.float32), np.asarray(bq, np.float32),
                     np.asarray(Wk, np.float32), np.asarray(bk, np.float32),
                     np.asarray(Wv, np.float32), np.asarray(bv, np.float32),
                     np.asarray(Wo, np.float32), np.asarray(bo, np.float32))
    return out
